# revision 10
# baseline (speedup 1.0000x reference)
"""Trainium2 Bass kernel for the GNN-RNN diagnoser.

Model (per reference): x [8192,1,5,128]; 128 sequential timesteps; each step
runs 5 per-node MLPs (130->256->256->64, relu/relu/tanh) where node inputs are
[x of 2 parents ; states of 2 parents]; then head 325->256 relu, BatchNorm
(training mode, full-batch stats), 256->7, softmax.

Strategy:
  - Data-parallel: batch 8192 -> 8 cores x 1024.
  - Feature-major on device: activations stored [features, batch].
  - States kept as 5 "parent pair" SBUF tensors P[m] = [s_p0 ; s_p1] (128 x B)
    in bf16, ping-ponged across steps, so layer-1 is a single K=128 matmul
    plus a K=2 matmul for the parent x values (accumulated in PSUM).
  - Loop runs in bf16 (fp32 PSUM accumulation); head matmul in fp32.
  - BatchNorm/Wo2/softmax tail done on host (needs cross-core batch stats;
    ~0.003%% of FLOPs).
"""

import numpy as np
import ml_dtypes
from contextlib import ExitStack

import concourse.bass as bass
import concourse.bacc as bacc
import concourse.tile as tile
from concourse import mybir
from concourse.bass_utils import run_bass_kernel_spmd

F32 = mybir.dt.float32
BF16 = mybir.dt.bfloat16
AF = mybir.ActivationFunctionType
ALU = mybir.AluOpType

NCORES = 8
B = 8192
BC = B // NCORES          # 1024 batch per core
T = 128
TC = 16                   # timesteps per xg ring-buffer chunk
NODE = 5
FML = 64
F0 = 256
NCS = 2                   # batch N-chunks of 512 (PSUM bank limit)
NW = BC // NCS            # 512

PARENTS = [(3, 4), (0, 4), (0, 1), (1, 2), (2, 3)]
BN_EPS = 1e-5

# For each node n, the pair-buffer slots holding s_n: (pair index m, row off)
SLOTS = {n: [] for n in range(NODE)}
for m, (p0, p1) in enumerate(PARENTS):
    SLOTS[p0].append((m, 0))
    SLOTS[p1].append((m, 64))


def _build_nc():
    nc = bacc.Bacc("TRN2", target_bir_lowering=False, debug=False,
                   num_devices=NCORES)

    # ---- DRAM I/O -------------------------------------------------------
    xg_d = nc.dram_tensor("xg", [10, T, BC], BF16, kind="ExternalInput")
    xt_d = nc.dram_tensor("xt", [NODE, BC], F32, kind="ExternalInput")
    w1s_d = nc.dram_tensor("w1s", [128, NODE * F0], BF16, kind="ExternalInput")
    w1x_d = nc.dram_tensor("w1x", [10, NODE * F0], BF16, kind="ExternalInput")
    w2s_d = nc.dram_tensor("w2s", [128, NODE * 2 * F0], BF16, kind="ExternalInput")
    w3s_d = nc.dram_tensor("w3s", [128, NODE * 2 * FML], BF16, kind="ExternalInput")
    b1_d = nc.dram_tensor("b1p", [128, NODE * 2], F32, kind="ExternalInput")
    b2_d = nc.dram_tensor("b2p", [128, NODE * 2], F32, kind="ExternalInput")
    b3_d = nc.dram_tensor("b3p", [64, NODE], F32, kind="ExternalInput")
    wo1a_d = nc.dram_tensor("wo1a", [NODE, F0], F32, kind="ExternalInput")
    wo1b_d = nc.dram_tensor("wo1b", [128, F0], F32, kind="ExternalInput")
    wo1c_d = nc.dram_tensor("wo1c", [128, F0], F32, kind="ExternalInput")
    wo1d_d = nc.dram_tensor("wo1d", [64, F0], F32, kind="ExternalInput")
    bo1_d = nc.dram_tensor("bo1p", [128, 2], F32, kind="ExternalInput")
    h0_d = nc.dram_tensor("h0", [128, BC], F32, kind="ExternalOutput")
    h1_d = nc.dram_tensor("h1", [128, BC], F32, kind="ExternalOutput")

    with tile.TileContext(nc) as tc, ExitStack() as ctx:
        const = ctx.enter_context(tc.tile_pool(name="const", bufs=1))
        spool = ctx.enter_context(tc.tile_pool(name="state", bufs=1))
        apool = ctx.enter_context(tc.tile_pool(name="act", bufs=3))
        xpool = ctx.enter_context(tc.tile_pool(name="xgr", bufs=3))
        hpool = ctx.enter_context(tc.tile_pool(name="head", bufs=1))
        ps1 = ctx.enter_context(tc.tile_pool(name="ps1", bufs=3, space="PSUM"))
        ps2 = ctx.enter_context(tc.tile_pool(name="ps2", bufs=3, space="PSUM"))
        ps3 = ctx.enter_context(tc.tile_pool(name="ps3", bufs=2, space="PSUM"))

        def load(pool, dram, shape, dtype, tag):
            t = pool.tile(shape, dtype, tag=tag, name=tag)
            nc.sync.dma_start(out=t[:], in_=dram[:])
            return t

        xt = load(const, xt_d, [NODE, BC], F32, "xt")
        w1s = load(const, w1s_d, [128, NODE * F0], BF16, "w1s")
        w1x = load(const, w1x_d, [10, NODE * F0], BF16, "w1x")
        w2s = load(const, w2s_d, [128, NODE * 2 * F0], BF16, "w2s")
        w3s = load(const, w3s_d, [128, NODE * 2 * FML], BF16, "w3s")
        b1 = load(const, b1_d, [128, NODE * 2], F32, "b1")
        b2 = load(const, b2_d, [128, NODE * 2], F32, "b2")
        b3 = load(const, b3_d, [64, NODE], F32, "b3")
        wo1a = load(const, wo1a_d, [NODE, F0], F32, "wo1a")
        wo1b = load(const, wo1b_d, [128, F0], F32, "wo1b")
        wo1c = load(const, wo1c_d, [128, F0], F32, "wo1c")
        wo1d = load(const, wo1d_d, [64, F0], F32, "wo1d")
        bo1 = load(const, bo1_d, [128, 2], F32, "bo1")

        # state pair buffers, ping-pong
        P = [[spool.tile([128, BC], BF16, tag=f"P{ph}_{m}", name=f"P{ph}_{m}") for m in range(NODE)]
             for ph in range(2)]
        for ph in range(2):
            for m in range(NODE):
                nc.vector.memset(P[ph][m][:], 0.0)

        xg_ring = {}

        def step(t_abs):
            cur = P[t_abs % 2]
            nxt = P[(t_abs + 1) % 2]
            c, tl = divmod(t_abs, TC)
            if tl == 0:
                xr = xpool.tile([10, TC, BC], BF16, tag="xgr", name="xgr")
                nc.sync.dma_start(out=xr[:], in_=xg_d[:, c * TC:(c + 1) * TC, :])
                xg_ring[c] = xr
            xr = xg_ring[c]
            for n in range(NODE):
                # ---- layer 1: [130] -> 256
                h1t = [apool.tile([128, BC], BF16, tag=f"h1_{mc}", name=f"h1_{mc}")
                       for mc in range(2)]
                for mc in range(2):
                    for ncs in range(NCS):
                        pt = ps1.tile([128, NW], F32, tag="ps1", name="ps1t")
                        nc.tensor.matmul(
                            pt[:],
                            lhsT=w1s[:, n * F0 + mc * 128: n * F0 + (mc + 1) * 128],
                            rhs=cur[n][:, ncs * NW:(ncs + 1) * NW],
                            start=True, stop=False)
                        nc.tensor.matmul(
                            pt[:],
                            lhsT=w1x[:, n * F0 + mc * 128: n * F0 + (mc + 1) * 128],
                            rhs=xr[:, tl, ncs * NW:(ncs + 1) * NW],
                            start=False, stop=True)
                        nc.vector.tensor_scalar(
                            h1t[mc][:, ncs * NW:(ncs + 1) * NW], pt[:],
                            b1[:, n * 2 + mc: n * 2 + mc + 1], 0.0,
                            ALU.add, ALU.max)
                # ---- layer 2: 256 -> 256
                h2t = [apool.tile([128, BC], BF16, tag=f"h2_{mc}", name=f"h2_{mc}")
                       for mc in range(2)]
                for mc in range(2):
                    for ncs in range(NCS):
                        pt = ps2.tile([128, NW], F32, tag="ps2", name="ps2t")
                        for kc in range(2):
                            nc.tensor.matmul(
                                pt[:],
                                lhsT=w2s[:, (n * 2 + kc) * F0 + mc * 128:
                                         (n * 2 + kc) * F0 + (mc + 1) * 128],
                                rhs=h1t[kc][:, ncs * NW:(ncs + 1) * NW],
                                start=(kc == 0), stop=(kc == 1))
                        nc.scalar.activation(
                            h2t[mc][:, ncs * NW:(ncs + 1) * NW], pt[:],
                            AF.Relu, bias=b2[:, n * 2 + mc: n * 2 + mc + 1])
                # ---- layer 3: 256 -> 64, tanh, scatter to pair slots
                (m1, o1), (m2, o2) = SLOTS[n]
                # pick a primary target whose offset can match the psum rows
                if o1 == 0 or o2 == 0:
                    if o1 != 0:
                        (m1, o1), (m2, o2) = (m2, o2), (m1, o1)
                    off = 0
                else:
                    off = 64   # node 4: both slots at row 64 -> col-tile psum
                for ncs in range(NCS):
                    pt = ps3.tile([128, NW], F32, tag="ps3", name="ps3t")
                    for kc in range(2):
                        nc.tensor.matmul(
                            pt[off:off + 64, :],
                            lhsT=w3s[:, (n * 2 + kc) * FML:(n * 2 + kc + 1) * FML],
                            rhs=h2t[kc][:, ncs * NW:(ncs + 1) * NW],
                            start=(kc == 0), stop=(kc == 1),
                            tile_position=(0, off))
                    nc.scalar.activation(
                        nxt[m1][o1:o1 + 64, ncs * NW:(ncs + 1) * NW],
                        pt[off:off + 64, :],
                        AF.Tanh, bias=b3[:, n:n + 1])
                # secondary copy
                if o2 == o1:
                    nc.vector.tensor_copy(out=nxt[m2][o2:o2 + 64, :],
                                          in_=nxt[m1][o1:o1 + 64, :])
                else:
                    nc.sync.dma_start(out=nxt[m2][o2:o2 + 64, :],
                                      in_=nxt[m1][o1:o1 + 64, :])

        for t_abs in range(T):
            step(t_abs)

        # ---- head: feat = [x_T(5); s0..s4(320)] -> 256, relu  (fp32)
        fin = P[T % 2]
        sf01 = hpool.tile([128, BC], F32, tag="sf01")
        sf23 = hpool.tile([128, BC], F32, tag="sf23")
        sf4b = hpool.tile([64, BC], BF16, tag="sf4b")
        sf4 = hpool.tile([64, BC], F32, tag="sf4")
        nc.vector.tensor_copy(out=sf01[:], in_=fin[2][:])    # [s0; s1]
        nc.vector.tensor_copy(out=sf23[:], in_=fin[4][:])    # [s2; s3]
        nc.sync.dma_start(out=sf4b[:], in_=fin[0][64:128, :])  # s4
        nc.vector.tensor_copy(out=sf4[:], in_=sf4b[:])
        hout = [hpool.tile([128, BC], F32, tag=f"hout{mc}", name=f"hout{mc}") for mc in range(2)]
        for mc in range(2):
            for ncs in range(NCS):
                pt = ps1.tile([128, NW], F32, tag="ps1", name="ps1t")
                sl = slice(ncs * NW, (ncs + 1) * NW)
                nc.tensor.matmul(pt[:], lhsT=wo1a[:, mc * 128:(mc + 1) * 128],
                                 rhs=xt[:, sl], start=True, stop=False)
                nc.tensor.matmul(pt[:], lhsT=wo1b[:, mc * 128:(mc + 1) * 128],
                                 rhs=sf01[:, sl], start=False, stop=False)
                nc.tensor.matmul(pt[:], lhsT=wo1c[:, mc * 128:(mc + 1) * 128],
                                 rhs=sf23[:, sl], start=False, stop=False)
                nc.tensor.matmul(pt[:], lhsT=wo1d[:, mc * 128:(mc + 1) * 128],
                                 rhs=sf4[:, sl], start=False, stop=True)
                nc.scalar.activation(hout[mc][:, sl], pt[:], AF.Relu,
                                     bias=bo1[:, mc:mc + 1])
        nc.sync.dma_start(out=h0_d[:], in_=hout[0][:])
        nc.sync.dma_start(out=h1_d[:], in_=hout[1][:])

    nc.compile()
    return nc


_NC = None


def _get_nc():
    global _NC
    if _NC is None:
        _NC = _build_nc()
    return _NC


def _prep_inputs(x, W1, b1, W2, b2, W3, b3):
    """Host-side packing of weights and the parent-gathered x sequence."""
    bf = ml_dtypes.bfloat16
    xs = x.reshape(B, NODE, T)                      # [B, node, t]

    # xg[2n + j, t, b] = x[b, parents[n][j], t]
    xg = np.empty((10, T, B), dtype=bf)
    for n in range(NODE):
        for j in range(2):
            xg[2 * n + j] = xs[:, PARENTS[n][j], :].T.astype(bf)
    xt = np.ascontiguousarray(xs[:, :, T - 1].T.astype(np.float32))  # [5, B]

    w1s = np.empty((128, NODE * F0), dtype=bf)
    # block "diagonal" x-part weights: rows 2n'+j match xg rows; only the
    # rows belonging to node n are nonzero in node n's column block.
    w1x = np.zeros((10, NODE * F0), dtype=bf)
    for n in range(NODE):
        w1s[:, n * F0:(n + 1) * F0] = W1[n, 2:130, :].astype(bf)
        w1x[2 * n:2 * n + 2, n * F0:(n + 1) * F0] = W1[n, 0:2, :].astype(bf)
    w2s = np.empty((128, NODE * 2 * F0), dtype=bf)
    w3s = np.empty((128, NODE * 2 * FML), dtype=bf)
    for n in range(NODE):
        for kc in range(2):
            w2s[:, (n * 2 + kc) * F0:(n * 2 + kc + 1) * F0] = \
                W2[n, kc * 128:(kc + 1) * 128, :].astype(bf)
            w3s[:, (n * 2 + kc) * FML:(n * 2 + kc + 1) * FML] = \
                W3[n, kc * 128:(kc + 1) * 128, :].astype(bf)
    b1p = np.empty((128, NODE * 2), dtype=np.float32)
    b2p = np.empty((128, NODE * 2), dtype=np.float32)
    b3p = np.empty((64, NODE), dtype=np.float32)
    for n in range(NODE):
        for mc in range(2):
            b1p[:, n * 2 + mc] = b1[n, mc * 128:(mc + 1) * 128]
            b2p[:, n * 2 + mc] = b2[n, mc * 128:(mc + 1) * 128]
        b3p[:, n] = b3[n]
    return xg, xt, w1s, w1x, w2s, w3s, b1p, b2p, b3p


def kernel(x, W1, b1, W2, b2, W3, b3, Wo1, bo1, gamma, beta, Wo2, bo2):
    x = np.asarray(x, dtype=np.float32)
    xg, xt, w1s, w1x, w2s, w3s, b1p, b2p, b3p = _prep_inputs(
        np.asarray(x), np.asarray(W1), np.asarray(b1), np.asarray(W2),
        np.asarray(b2), np.asarray(W3), np.asarray(b3))

    Wo1 = np.asarray(Wo1, dtype=np.float32)
    wo1a = np.ascontiguousarray(Wo1[0:5, :])
    wo1b = np.ascontiguousarray(Wo1[5:133, :])
    wo1c = np.ascontiguousarray(Wo1[133:261, :])
    wo1d = np.ascontiguousarray(Wo1[261:325, :])
    bo1p = np.asarray(bo1, dtype=np.float32).reshape(2, 128).T.copy()

    shared = dict(w1s=w1s, w1x=w1x, w2s=w2s, w3s=w3s, b1p=b1p, b2p=b2p,
                  b3p=b3p, wo1a=wo1a, wo1b=wo1b, wo1c=wo1c, wo1d=wo1d,
                  bo1p=bo1p)
    in_maps = []
    for c in range(NCORES):
        sl = slice(c * BC, (c + 1) * BC)
        in_maps.append(dict(shared, xg=np.ascontiguousarray(xg[:, :, sl]),
                            xt=np.ascontiguousarray(xt[:, sl])))

    nc = _get_nc()
    res = run_bass_kernel_spmd(nc, in_maps, core_ids=list(range(NCORES)))

    # gather h = relu(feat @ Wo1 + bo1), shape [8192, 256]
    h = np.empty((B, 256), dtype=np.float32)
    for c, r in enumerate(res.results):
        sl = slice(c * BC, (c + 1) * BC)
        h[sl, 0:128] = r["h0"].T
        h[sl, 128:256] = r["h1"].T

    # ---- host tail: BatchNorm (training-mode batch stats) + Wo2 + softmax
    mu = h.mean(axis=0)
    var = ((h - mu) ** 2).mean(axis=0)
    hn = (h - mu) / np.sqrt(var + BN_EPS) * np.asarray(gamma) + np.asarray(beta)
    logits = hn @ np.asarray(Wo2) + np.asarray(bo2)
    e = np.exp(logits - logits.max(axis=1, keepdims=True))
    return (e / e.sum(axis=1, keepdims=True)).astype(np.float32)
